# revision 14
# baseline (speedup 1.0000x reference)
"""CircleLossV2 on 8 Trainium2 NeuronCores (Bass/Tile) — symmetric triangle.

Strategy:
  - Host: sort rows by label; per-core rotate by 512*k cols so each core owns
    local tile-rows {0..3, 32..35} of its rotated copy (identical program
    across cores = SPMD).  sim is symmetric: tile-row i computes only tiles
    [i, i+W) mod 64 (W=33 for i<32 else 32) — every unordered tile pair
    exactly once, 260 of 512 tiles per core.
  - Host normalizes embeddings (fp64) and ships eT in an extended layout
    [128, 8704] bf16 (left pad 64 for the pos window, wrap so every row's
    range is contiguous).
  - Device, phase-major (phases of two 1024-col ext chunks): per owned-row
    group: bf16 matmuls -> PSUM fp32 sim (diag fixed by a -2I matmul);
    (s+0.75)^2 via ACT-Square(bias) fp32 or fp16 DVE ts-add + tt-mult (some
    mults on GPSIMD); one merged exp(64*u2-140) per row-phase on ACT with
    accum_out row sums (bf16 E out); column sums via ones-matmul into PSUM
    [1,1024] per chunk (symmetric contribution to rows of the column
    blocks), evacuated once per chunk.
  - Each row's first 192 cols (the same-class window overlap) go through
    the ACT fp32 path; the host subtracts same-class terms computed in fp64
    from the DMA'd window sim pw [128,256] (bit-identical PE values), so
    cancellation noise is ~1e-6.
  - Host epilogue: row pieces + colsums - CR; pos term from pw; softplus.
"""

import sys

sys.path.insert(0, "/opt/trn_rl_repo")

import numpy as np
from ml_dtypes import bfloat16

import concourse.bass as bass
import concourse.bacc as bacc
import concourse.mybir as mybir
import concourse.tile as tile
from concourse.bass_utils import run_bass_kernel_spmd

F32 = mybir.dt.float32
F16 = mybir.dt.float16
BF16 = mybir.dt.bfloat16
AF = mybir.ActivationFunctionType
OP = mybir.AluOpType

B = 8192
D = 128
NCORES = 8
EXT_OFF = 64
EXT_W = 8704
NCHUNK = 9
MHN = 140.0
MHP = 100.0
LOCAL_ROWS = [0, 1, 2, 3, 32, 33, 34, 35]
PREF = 192  # per-row fp32-ACT prefix (window overlap region)

_PROG = None


def _wtiles(i):
    return 33 if i < 32 else 32


def _build_schedule():
    """Groups of same-row pieces within a 2-chunk phase, plus bookkeeping."""
    row_pieces = []
    for r_idx, i in enumerate(LOCAL_ROWS):
        lo = EXT_OFF + i * 128
        hi = lo + _wtiles(i) * 128
        ps = []
        a = lo
        while a < hi:
            b = min(hi, 1024 * (a // 1024 + 1))
            ps.append(dict(r=r_idx, i=i, a=a, b=b, lo=lo, jc=a // 1024,
                           first=(a == lo)))
            a = b
        row_pieces.append(ps)

    groups = []
    for ph in range(5):
        chunks = (2 * ph, 2 * ph + 1)
        # singles (1-piece groups) first so their chunk completes early
        cand = []
        for r_idx in range(8):
            sel = [p for p in row_pieces[r_idx] if p["jc"] in chunks]
            if sel:
                cand.append(sel)
        cand.sort(key=lambda s: len(s))
        for sel in cand:
            groups.append(dict(pieces=sel, ph=ph))

    # segments per group: fp32-ACT prefix (PREF cols) for first pieces, and
    # one merged fp16/fp32 segment for the rest of the group
    slot = 0
    for g in groups:
        segs = []
        p0 = g["pieces"][0]
        if p0["first"]:
            segs.append(dict(kind="pref", a=p0["a"], b=p0["a"] + PREF,
                             slot=slot))
            slot += 1
            rest_a = p0["a"] + PREF
        else:
            rest_a = p0["a"]
        rest_b = g["pieces"][-1]["b"]
        if rest_a < rest_b:
            segs.append(dict(kind="main", a=rest_a, b=rest_b, slot=slot))
            slot += 1
        g["segs"] = segs
    nslot = slot

    # colsum sub-matmuls per group (lagged one group at emission):
    # split [max(a, lo+128), b) of the whole group range by 512 banks
    order = []
    for gi, g in enumerate(groups):
        p0 = g["pieces"][0]
        cs_a = max(p0["a"], p0["lo"] + 128)
        cs_b = g["pieces"][-1]["b"]
        subs = []
        gpos = cs_a
        while gpos < cs_b:
            g1 = min(cs_b, 512 * (gpos // 512 + 1))
            subs.append((gpos, g1))
            order.append((gi, gpos, g1))
            gpos = g1
        g["cs_subs"] = subs
    bank_first, bank_last = {}, {}
    for gi, g0, g1 in order:
        key = (g0 // 512)
        if key not in bank_first:
            bank_first[key] = (gi, g0)
        bank_last[key] = (gi, g0)
    # chunk -> last group touching it (for evac placement)
    chunk_last = {}
    for gi, g in enumerate(groups):
        for p in g["pieces"]:
            chunk_last[p["jc"]] = gi
    return groups, nslot, bank_first, bank_last, chunk_last


GROUPS, NSLOT, BANK_FIRST, BANK_LAST, CHUNK_LAST = _build_schedule()
NGRP = len(GROUPS)

EVAC_ENG = {jc: ("act" if jc in (0, 4) else "dve") for jc in range(NCHUNK)}


def _assign_engines():
    """All main-segment squares: DVE ts-add (PSUM drain) + GPSIMD tt-mult.
    Measured: GPSIMD fp16 tt ~1.0 ns/col, far under DVE/ACT loads, so the
    mult pass goes entirely to the otherwise-idle GPSIMD."""
    for g in GROUPS:
        g["eng"] = "gps" if g["segs"][-1]["kind"] == "main" else "act"
    return (0.0, 0.0, 0.0)


def _assign_engines_greedy_unused():
    act = 1283.0 + 185.0 * NSLOT + 8 * 303.0 + 2 * 997.0  # table+accum+pref+evac
    for g in GROUPS:
        for seg in g["segs"]:
            act += (224.0 + (seg["b"] - seg["a"])) / 1.2  # exps
    dve = 7 * 1192.0 + 8 * 392.0  # evacs + pw copies
    gps = 1000.0
    order = sorted(GROUPS, key=lambda g: -(g["segs"][-1]["b"] - g["segs"][-1]["a"]))
    for g in order:
        seg = g["segs"][-1]
        if seg["kind"] != "main":
            g["eng"] = "act"
            continue
        w = float(seg["b"] - seg["a"])
        np_ = len(g["pieces"])
        c_act = (np_ * 172 + w) / 1.2
        c_dve = (np_ * 178 + 1.5 * w) / 0.96
        c_gps_d = (np_ * 120 + w) / 0.96
        c_gps_g = np_ * 240 + 2.34 * w
        best, bcost = None, None
        for eng, (na, nd, ng) in (
            ("act", (act + c_act, dve, gps)),
            ("dve", (act, dve + c_dve, gps)),
            ("gps", (act, dve + c_gps_d, gps + c_gps_g)),
        ):
            m = max(na, nd, ng)
            if bcost is None or m < bcost:
                best, bcost = eng, m
        g["eng"] = best
        if best == "act":
            act += c_act
        elif best == "dve":
            dve += c_dve
        else:
            dve += c_gps_d
            gps += c_gps_g

    # iterative improvement: single-group moves that lower the max
    def costs(g):
        seg = g["segs"][-1]
        w = float(seg["b"] - seg["a"])
        np_ = len(g["pieces"])
        return {
            "act": ((np_ * 172 + w) / 1.2, 0.0, 0.0),
            "dve": (0.0, (np_ * 178 + 1.5 * w) / 0.96, 0.0),
            "gps": (0.0, (np_ * 120 + w) / 0.96, np_ * 240 + 2.0 * w),
        }
    for _ in range(100):
        improved = False
        for g in GROUPS:
            if g["segs"][-1]["kind"] != "main":
                continue
            cc = costs(g)
            cur = g["eng"]
            for eng in ("act", "dve", "gps"):
                if eng == cur:
                    continue
                na = act - cc[cur][0] + cc[eng][0]
                nd = dve - cc[cur][1] + cc[eng][1]
                ng = gps - cc[cur][2] + cc[eng][2]
                if max(na, nd, ng) < max(act, dve, gps) - 100:
                    act, dve, gps = na, nd, ng
                    g["eng"] = eng
                    improved = True
        if not improved:
            break
    return act, dve, gps


_PRED = _assign_engines()


def _register_const(nc, val, dtype=F32):
    t = nc.alloc_sbuf_tensor(f"uconst-{dtype.name}-{val}", [128, 1], dtype)
    nc.gpsimd.memset(t.ap(), val)
    nc.const_aps.aps[(dtype, val)] = t.ap()


def _build():
    nc = bacc.Bacc("TRN2", target_bir_lowering=False, debug=False, num_devices=NCORES)
    for v in (0.75, -MHN):
        _register_const(nc, v)
    nc.all_engine_barrier()

    xt_in = nc.dram_tensor("xt", [D, EXT_W], BF16, kind="ExternalInput")
    i128_in = nc.dram_tensor("i128", [128, 128], BF16, kind="ExternalInput")
    n2i_in = nc.dram_tensor("n2i", [128, 128], BF16, kind="ExternalInput")
    ones_in = nc.dram_tensor("ones128", [128, 1], BF16, kind="ExternalInput")
    stats_out = nc.dram_tensor("stats", [128, NSLOT], F32, kind="ExternalOutput")
    cs_out = nc.dram_tensor("cs", [1, NCHUNK * 1024], F32, kind="ExternalOutput")
    pw_out = nc.dram_tensor("pw", [8, 128, 256], F32, kind="ExternalOutput")

    with tile.TileContext(nc) as tc:
        with (
            tc.tile_pool(name="cst", bufs=1) as cst,
            tc.tile_pool(name="sbq", bufs=2) as sbq,   # fp32 u2 (ACT path)
            tc.tile_pool(name="sbh", bufs=2) as sbh,   # fp16 u / u2
            tc.tile_pool(name="sbe", bufs=3) as sbe,   # bf16 E
            tc.tile_pool(name="psd", bufs=2, space="PSUM") as psd,
            tc.tile_pool(name="psc", bufs=2, space="PSUM") as psc,
        ):
            # ---------------- constants ----------------
            i128s = cst.tile([128, 128], BF16, tag="i128s", name="i128s")
            nc.sync.dma_start(i128s[:], i128_in.ap())
            i128 = cst.tile([128, 128], BF16, tag="i128", name="i128")
            nc.vector.tensor_copy(i128[:], i128s[:])

            n2is = cst.tile([128, 128], BF16, tag="n2is", name="n2is")
            nc.sync.dma_start(n2is[:], n2i_in.ap())
            n2i = cst.tile([128, 128], BF16, tag="n2i", name="n2i")
            nc.vector.tensor_copy(n2i[:], n2is[:])

            ones_s = cst.tile([128, 1], BF16, tag="oness", name="ones_s")
            nc.sync.dma_start(ones_s[:], ones_in.ap())
            ones_a = cst.tile([128, 1], BF16, tag="onesa", name="ones_a")
            nc.vector.tensor_copy(ones_a[:], ones_s[:])

            xt = cst.tile([128, EXT_W], BF16, tag="xt", name="xt")
            for jc in range(NCHUNK):
                c0, c1 = jc * 1024, min((jc + 1) * 1024, EXT_W)
                nc.sync.dma_start(xt[:, c0:c1], xt_in.ap()[:, c0:c1])

            NS = cst.tile([128, NSLOT], F32, tag="NS", name="NS")
            css = cst.tile([1, NCHUNK * 1024], F32, tag="css", name="css")

            state = {}

            def emit_sims(g):
                lo = g["pieces"][0]["lo"]
                lhsT = xt[:, lo: lo + 128]
                for p in g["pieces"]:
                    L = p["b"] - p["a"]
                    ps_t = psd.tile([128, 1024], F32, tag="ps",
                                    name=f"ps_{p['a']}")
                    for s0 in range(0, L, 512):
                        s1 = min(s0 + 512, L)
                        has_diag = p["first"] and s0 == 0
                        nc.tensor.matmul(
                            ps_t[:, s0:s1], lhsT,
                            xt[:, p["a"] + s0: p["a"] + s1],
                            start=True, stop=not has_diag,
                        )
                        if has_diag:
                            nc.tensor.matmul(
                                ps_t[:, 0:128], n2i[:], i128[:],
                                start=False, stop=True, skip_group_check=True,
                            )
                    state[("ps", p["r"], p["a"])] = ps_t

            def emit_squares(g):
                eng = g["eng"]
                ga = g["segs"][0]["a"] if g["segs"][0]["kind"] == "pref" else None
                main = g["segs"][-1]
                if main["kind"] == "main":
                    u2h = sbh.tile([128, 2048], F16, tag="u2h",
                                   name=f"u2h{main['slot']}")
                    state[("u2h", id(g))] = u2h
                if eng == "act" and main["kind"] == "main":
                    u2f = sbq.tile([128, 2048], F32, tag="u2f",
                                   name=f"u2f{main['slot']}")
                    state[("u2f", id(g))] = u2f
                for p in g["pieces"]:
                    ps_t = state[("ps", p["r"], p["a"])]
                    L = p["b"] - p["a"]
                    s0 = 0
                    if p["first"]:
                        u2a = sbq.tile([128, 256], F32, tag="u2a",
                                       name=f"u2a{p['r']}")
                        nc.scalar.activation(
                            u2a[:, 0:PREF], ps_t[:, 0:PREF], AF.Square,
                            bias=0.75,
                        )
                        state[("u2a", id(g))] = u2a
                        s0 = PREF
                    off = p["a"] + s0 - main["a"]
                    w = L - s0
                    if w <= 0:
                        continue
                    if eng == "act":
                        u2f = state[("u2f", id(g))]
                        nc.scalar.activation(
                            u2f[:, off: off + w], ps_t[:, s0:L], AF.Square,
                            bias=0.75,
                        )
                    else:
                        u2h = state[("u2h", id(g))]
                        uh = sbh.tile([128, 1024], F16, tag="uh",
                                      name=f"uh{p['a']}")
                        nc.vector.tensor_scalar(
                            uh[:, 0:w], ps_t[:, s0:L], 0.75, None, OP.add
                        )
                        mul_eng = nc.vector if eng == "dve" else nc.gpsimd
                        mul_eng.tensor_tensor(
                            u2h[:, off: off + w], uh[:, 0:w], uh[:, 0:w],
                            op=OP.mult,
                        )
                for p in g["pieces"]:
                    state.pop(("ps", p["r"], p["a"]))

            def emit_exp(g):
                ga, gb = g["pieces"][0]["a"], g["pieces"][-1]["b"]
                E = sbe.tile([128, 2048], BF16, tag="E", name=f"E{id(g)}")
                for seg in g["segs"]:
                    if seg["kind"] == "pref":
                        src = state.pop(("u2a", id(g)))[:, 0:PREF]
                    else:
                        w = seg["b"] - seg["a"]
                        if g["eng"] == "act":
                            src = state.pop(("u2f", id(g)))[:, 0:w]
                        else:
                            src = state.pop(("u2h", id(g)))[:, 0:w]
                    nc.scalar.activation(
                        E[:, seg["a"] - ga: seg["b"] - ga], src, AF.Exp,
                        bias=-MHN, scale=64.0,
                        accum_out=NS[:, seg["slot"]: seg["slot"] + 1],
                    )
                state[("E", id(g))] = E


            def emit_colsum(g, gi):
                if not g["cs_subs"]:
                    state.pop(("E", id(g)))
                    return
                ga = g["pieces"][0]["a"]
                E = state.pop(("E", id(g)))
                for (g0, g1) in g["cs_subs"]:
                    jc = g0 // 1024
                    cs_t = state.get(("cs", jc))
                    if cs_t is None:
                        cs_t = psc.tile([1, 1024], F32, tag="cs",
                                        name=f"cs{jc}")
                        state[("cs", jc)] = cs_t
                    bank = g0 // 512
                    nc.tensor.matmul(
                        cs_t[0:1, g0 - jc * 1024: g1 - jc * 1024],
                        ones_a[:],
                        E[:, g0 - ga: g1 - ga],
                        start=BANK_FIRST[bank] == (gi, g0),
                        stop=BANK_LAST[bank] == (gi, g0),
                        skip_group_check=True,
                    )

            def emit_evac(jc):
                cs_t = state.pop(("cs", jc), None)
                if cs_t is None:
                    return
                c0 = jc * 1024
                if EVAC_ENG[jc] == "act":
                    nc.scalar.copy(css[0:1, c0: c0 + 1024], cs_t[0:1, :])
                else:
                    nc.vector.tensor_copy(css[0:1, c0: c0 + 1024], cs_t[0:1, :])

            def emit_pw(r_idx):
                i = LOCAL_ROWS[r_idx]
                lo = EXT_OFF + i * 128
                pw = psd.tile([128, 1024], F32, tag="ps", name=f"pw{r_idx}")
                nc.tensor.matmul(
                    pw[:, 0:256], xt[:, lo: lo + 128],
                    xt[:, lo - 64: lo + 192], start=True, stop=True,
                )
                pws = cst.tile([128, 256], F32, tag=f"pws{r_idx}",
                               name=f"pws{r_idx}")
                nc.vector.tensor_copy(pws[:], pw[:, 0:256])
                nc.sync.dma_start(pw_out.ap()[r_idx, :, :], pws[:])

            # pipelined emission: sims(G), squares(G), exp(G-1), colsum(G-2)
            # window sims spread mid-stream (chunks 0-4 landed by step 12)
            evac_due = {}
            for jc, gi in CHUNK_LAST.items():
                evac_due.setdefault(gi, []).append(jc)
            for step in range(NGRP + 2):
                if step < NGRP:
                    emit_sims(GROUPS[step])
                    emit_squares(GROUPS[step])
                if 12 <= step < 20:
                    emit_pw(step - 12)
                if 1 <= step < NGRP + 1:
                    emit_exp(GROUPS[step - 1])
                if step >= 2:
                    gi = step - 2
                    emit_colsum(GROUPS[gi], gi)
                    for jc in sorted(evac_due.get(gi, [])):
                        emit_evac(jc)

            # ---------------- writeback ----------------
            nc.sync.dma_start(stats_out.ap()[:], NS[:])
            nc.sync.dma_start(cs_out.ap()[:], css[:])

    nc.compile()
    return nc


def _get_prog():
    global _PROG
    if _PROG is None:
        _PROG = _build()
    return _PROG


def _prepare_inputs(embeddings, labels):
    x = np.asarray(embeddings, dtype=np.float32)
    lab = np.asarray(labels)
    assert x.shape == (B, D) and lab.shape == (B,)

    perm = np.argsort(lab, kind="stable")
    xs = x[perm]
    ls = lab[perm]

    _, inv_idx, counts = np.unique(ls, return_inverse=True, return_counts=True)
    cnt_row = counts[inv_idx]
    valid_sorted = (cnt_row >= 2) & (B - cnt_row >= 1)
    assert counts.max() <= 64, "window of 256 requires class size <= 64"

    e = xs / np.linalg.norm(xs.astype(np.float64), axis=1, keepdims=True).astype(
        np.float32
    )
    eT = np.ascontiguousarray(e.T)

    ident = np.eye(128, dtype=bfloat16)
    n2i = (-2.0 * np.eye(128)).astype(bfloat16)
    ones128 = np.ones((128, 1), dtype=bfloat16)

    ext_src = (np.arange(EXT_W) - EXT_OFF) % B
    in_maps = []
    for k in range(NCORES):
        sh = 512 * k
        rot_cols = (ext_src + sh) % B
        xt = np.ascontiguousarray(eT[:, rot_cols]).astype(bfloat16)
        in_maps.append(
            {"xt": xt, "i128": ident, "n2i": n2i, "ones128": ones128}
        )
    return in_maps, valid_sorted, ls


def _epilogue(results, valid_sorted, ls):
    NEG = np.zeros(B)
    CRv = np.zeros(B)
    PSv = np.zeros(B)
    cs_lo, cs_hi = 192, EXT_OFF + (35 + 32) * 128  # [192, 8640)
    ccols = np.arange(cs_lo, cs_hi)
    prow = np.arange(128)
    for k in range(NCORES):
        st = np.asarray(results[k]["stats"], dtype=np.float64)
        cs = np.asarray(results[k]["cs"], dtype=np.float64)[0]
        pw = np.asarray(results[k]["pw"], dtype=np.float64)
        sh = 512 * k
        for g in GROUPS:
            i = g["pieces"][0]["i"]
            rows = (sh + i * 128 + prow) % B
            for seg in g["segs"]:
                NEG[rows] += st[:, seg["slot"]]
        np.add.at(NEG, (ccols - EXT_OFF + sh) % B, cs[ccols])
        for r_idx, i in enumerate(LOCAL_ROWS):
            rows = (sh + i * 128 + prow) % B
            lr_rows = ls[rows]
            wcols = (sh + i * 128 - 64 + np.arange(256)) % B
            eq = lr_rows[:, None] == ls[wcols][None, :]
            eq[prow, 64 + prow] = False
            s = pw[r_idx]
            u2w = (s + 0.75) ** 2
            v2 = (s - 0.75) ** 2
            CRv[rows] += np.where(eq, np.exp(64.0 * u2w - MHN), 0.0).sum(axis=1)
            PSv[rows] += np.where(eq, np.exp(64.0 * v2 - MHP), 0.0).sum(axis=1)

    neg = np.maximum(NEG - CRv, 1e-250)
    with np.errstate(divide="ignore", invalid="ignore"):
        negterm = np.log(neg) + MHN
        posterm = np.log(np.maximum(PSv, 1e-250)) + MHP
    per_row = np.logaddexp(0.0, negterm + posterm)
    per_row = np.where(valid_sorted, per_row, 0.0)
    count = int(valid_sorted.sum())
    return np.float32(per_row.sum() / max(count, 1))


def kernel(embeddings, labels, _trace=False):
    nc = _get_prog()
    in_maps, valid_sorted, ls = _prepare_inputs(embeddings, labels)
    res = run_bass_kernel_spmd(
        nc, in_maps, core_ids=list(range(NCORES)), trace=_trace
    )
    loss = _epilogue(res.results, valid_sorted, ls)
    if _trace:
        return loss, res
    return loss


# revision 18
# speedup vs baseline: 1.1924x; 1.1924x over previous
"""CircleLossV2 on 8 Trainium2 NeuronCores (Bass/Tile) — symmetric triangle.

Strategy:
  - Host: sort rows by label; per-core rotate by 512*k cols so each core owns
    local tile-rows {0..3, 32..35} of its rotated copy (identical program
    across cores = SPMD).  sim is symmetric: tile-row i computes only tiles
    [i, i+W) mod 64 (W=33 for i<32 else 32) — every unordered tile pair
    exactly once, 260 of 512 tiles per core.
  - Host normalizes embeddings (fp64) and ships eT in an extended layout
    [128, 8704] bf16 (left pad 64 for the pos window, wrap so every row's
    range is contiguous).
  - Device, phase-major (phases of two 1024-col ext chunks): per owned-row
    group: bf16 matmuls -> PSUM fp32 sim (diag fixed by a -2I matmul);
    (s+0.75)^2 via ACT-Square(bias) fp32 or fp16 DVE ts-add + tt-mult (some
    mults on GPSIMD); one merged exp(64*u2-140) per row-phase on ACT with
    accum_out row sums (bf16 E out); column sums via ones-matmul into PSUM
    [1,1024] per chunk (symmetric contribution to rows of the column
    blocks), evacuated once per chunk.
  - Each row's first 192 cols (the same-class window overlap) go through
    the ACT fp32 path; the host subtracts same-class terms computed in fp64
    from the DMA'd window sim pw [128,256] (bit-identical PE values), so
    cancellation noise is ~1e-6.
  - Host epilogue: row pieces + colsums - CR; pos term from pw; softplus.
"""

import sys

sys.path.insert(0, "/opt/trn_rl_repo")

import numpy as np
from ml_dtypes import bfloat16

import concourse.bass as bass
import concourse.bacc as bacc
import concourse.mybir as mybir
import concourse.tile as tile
from concourse.bass_utils import run_bass_kernel_spmd

F32 = mybir.dt.float32
F16 = mybir.dt.float16
BF16 = mybir.dt.bfloat16
AF = mybir.ActivationFunctionType
OP = mybir.AluOpType

B = 8192
D = 128
NCORES = 8
EXT_OFF = 64
EXT_W = 8704
NCHUNK = 9
MHN = 140.0
MHP = 100.0
LOCAL_ROWS = [0, 1, 2, 3, 32, 33, 34, 35]
PREF = 192  # per-row fp32-ACT prefix (window overlap region)

_PROG = None


def _wtiles(i):
    return 33 if i < 32 else 32


def _build_schedule():
    """Groups of same-row pieces within a 2-chunk phase, plus bookkeeping."""
    row_pieces = []
    for r_idx, i in enumerate(LOCAL_ROWS):
        lo = EXT_OFF + i * 128
        hi = lo + _wtiles(i) * 128
        ps = []
        a = lo
        while a < hi:
            b = min(hi, 1024 * (a // 1024 + 1))
            ps.append(dict(r=r_idx, i=i, a=a, b=b, lo=lo, jc=a // 1024,
                           first=(a == lo)))
            a = b
        row_pieces.append(ps)

    groups = []
    for ph in range(5):
        chunks = (2 * ph, 2 * ph + 1)
        # singles (1-piece groups) first so their chunk completes early
        cand = []
        for r_idx in range(8):
            sel = [p for p in row_pieces[r_idx] if p["jc"] in chunks]
            if sel:
                cand.append(sel)
        cand.sort(key=lambda s: len(s))
        for sel in cand:
            groups.append(dict(pieces=sel, ph=ph))

    # segments per group: fp32-ACT prefix (PREF cols) for first pieces, and
    # one merged fp16/fp32 segment for the rest of the group
    slot = 0
    for g in groups:
        segs = []
        p0 = g["pieces"][0]
        if p0["first"]:
            segs.append(dict(kind="pref", a=p0["a"], b=p0["a"] + PREF,
                             slot=slot))
            slot += 1
            rest_a = p0["a"] + PREF
        else:
            rest_a = p0["a"]
        rest_b = g["pieces"][-1]["b"]
        if rest_a < rest_b:
            segs.append(dict(kind="main", a=rest_a, b=rest_b, slot=slot))
            slot += 1
        g["segs"] = segs
    nslot = slot

    # colsum sub-matmuls per group (lagged one group at emission):
    # split [max(a, lo+128), b) of the whole group range by 512 banks
    order = []
    for gi, g in enumerate(groups):
        p0 = g["pieces"][0]
        cs_a = max(p0["a"], p0["lo"] + 128)
        cs_b = g["pieces"][-1]["b"]
        subs = []
        gpos = cs_a
        while gpos < cs_b:
            g1 = min(cs_b, 512 * (gpos // 512 + 1))
            subs.append((gpos, g1))
            order.append((gi, gpos, g1))
            gpos = g1
        g["cs_subs"] = subs
    bank_first, bank_last = {}, {}
    for gi, g0, g1 in order:
        key = (g0 // 512)
        if key not in bank_first:
            bank_first[key] = (gi, g0)
        bank_last[key] = (gi, g0)
    # chunk -> last group touching it (for evac placement)
    chunk_last = {}
    for gi, g in enumerate(groups):
        for p in g["pieces"]:
            chunk_last[p["jc"]] = gi
    return groups, nslot, bank_first, bank_last, chunk_last


GROUPS, NSLOT, BANK_FIRST, BANK_LAST, CHUNK_LAST = _build_schedule()
NGRP = len(GROUPS)

EVAC_ENG = {jc: ("act" if jc in (0, 4) else "dve") for jc in range(NCHUNK)}


def _assign_engines():
    act = 1283.0 + 185.0 * NSLOT + 8 * 303.0 + 2 * 997.0  # table+accum+pref+evac
    for g in GROUPS:
        for seg in g["segs"]:
            act += (224.0 + (seg["b"] - seg["a"])) / 1.2  # exps
    dve = 7 * 1192.0 + 8 * 392.0  # evacs + pw copies
    gps = 1000.0
    order = sorted(GROUPS, key=lambda g: -(g["segs"][-1]["b"] - g["segs"][-1]["a"]))
    for g in order:
        seg = g["segs"][-1]
        if seg["kind"] != "main":
            g["eng"] = "act"
            continue
        w = float(seg["b"] - seg["a"])
        np_ = len(g["pieces"])
        c_act = (np_ * 172 + w) / 1.2
        c_dve = (np_ * 178 + 1.5 * w) / 0.96
        c_gps_d = (np_ * 120 + w) / 0.96
        c_gps_g = np_ * 240 + 2.3 * w
        best, bcost = None, None
        for eng, (na, nd, ng) in (
            ("act", (act + c_act, dve, gps)),
            ("dve", (act, dve + c_dve, gps)),
            ("gps", (act, dve + c_gps_d, gps + c_gps_g)),
        ):
            m = max(na, nd, ng)
            if bcost is None or m < bcost:
                best, bcost = eng, m
        g["eng"] = best
        if best == "act":
            act += c_act
        elif best == "dve":
            dve += c_dve
        else:
            dve += c_gps_d
            gps += c_gps_g

    # iterative improvement: single-group moves that lower the max
    def costs(g):
        seg = g["segs"][-1]
        w = float(seg["b"] - seg["a"])
        np_ = len(g["pieces"])
        return {
            "act": ((np_ * 172 + w) / 1.2, 0.0, 0.0),
            "dve": (0.0, (np_ * 178 + 1.5 * w) / 0.96, 0.0),
            "gps": (0.0, (np_ * 120 + w) / 0.96, np_ * 240 + 2.0 * w),
        }
    for _ in range(100):
        improved = False
        for g in GROUPS:
            if g["segs"][-1]["kind"] != "main":
                continue
            cc = costs(g)
            cur = g["eng"]
            for eng in ("act", "dve", "gps"):
                if eng == cur:
                    continue
                na = act - cc[cur][0] + cc[eng][0]
                nd = dve - cc[cur][1] + cc[eng][1]
                ng = gps - cc[cur][2] + cc[eng][2]
                if max(na, nd, ng) < max(act, dve, gps) - 100:
                    act, dve, gps = na, nd, ng
                    g["eng"] = eng
                    improved = True
        if not improved:
            break

    # interleave engines across consecutive groups within each phase so no
    # engine sees a long run of back-to-back heavy steps
    by_ph = {}
    for g in GROUPS:
        by_ph.setdefault(g["ph"], []).append(g)
    for ph, gs in by_ph.items():
        engs = [g["eng"] for g in gs]
        engs.sort(key=lambda e: {"dve": 0, "gps": 1, "act": 2}[e])
        # round-robin pick from the multiset: dve/gps/act rotating
        seq = []
        pools = {e: [x for x in engs if x == e] for e in ("dve", "gps", "act")}
        order = ("dve", "gps", "act")
        idx = 0
        while any(pools.values()):
            for _ in range(3):
                e = order[idx % 3]
                idx += 1
                if pools[e]:
                    seq.append(pools[e].pop())
                    break
            else:
                for e in order:
                    if pools[e]:
                        seq.append(pools[e].pop())
                        break
        for g, e in zip(gs, seq):
            g["eng"] = e
    return act, dve, gps


_PRED = _assign_engines()


def _register_const(nc, val, dtype=F32):
    t = nc.alloc_sbuf_tensor(f"uconst-{dtype.name}-{val}", [128, 1], dtype)
    nc.gpsimd.memset(t.ap(), val)
    nc.const_aps.aps[(dtype, val)] = t.ap()


def _build():
    nc = bacc.Bacc("TRN2", target_bir_lowering=False, debug=False, num_devices=NCORES)
    for v in (0.75, -MHN):
        _register_const(nc, v)
    nc.all_engine_barrier()

    xt_in = nc.dram_tensor("xt", [D, EXT_W], BF16, kind="ExternalInput")
    i128_in = nc.dram_tensor("i128", [128, 128], BF16, kind="ExternalInput")
    n2i_in = nc.dram_tensor("n2i", [128, 128], BF16, kind="ExternalInput")
    ones_in = nc.dram_tensor("ones128", [128, 1], BF16, kind="ExternalInput")
    stats_out = nc.dram_tensor("stats", [128, NSLOT], F32, kind="ExternalOutput")
    cs_out = nc.dram_tensor("cs", [1, NCHUNK * 1024], F32, kind="ExternalOutput")
    pw_out = nc.dram_tensor("pw", [8, 128, 256], F32, kind="ExternalOutput")

    with tile.TileContext(nc) as tc:
        with (
            tc.tile_pool(name="cst", bufs=1) as cst,
            tc.tile_pool(name="sbq", bufs=2) as sbq,   # fp32 u2 (ACT path)
            tc.tile_pool(name="sbh", bufs=2) as sbh,   # fp16 u / u2
            tc.tile_pool(name="sbe", bufs=3) as sbe,   # bf16 E
            tc.tile_pool(name="psd", bufs=2, space="PSUM") as psd,
            tc.tile_pool(name="psc", bufs=2, space="PSUM") as psc,
        ):
            # ---------------- constants ----------------
            i128s = cst.tile([128, 128], BF16, tag="i128s", name="i128s")
            nc.sync.dma_start(i128s[:], i128_in.ap())
            i128 = cst.tile([128, 128], BF16, tag="i128", name="i128")
            nc.vector.tensor_copy(i128[:], i128s[:])

            n2is = cst.tile([128, 128], BF16, tag="n2is", name="n2is")
            nc.sync.dma_start(n2is[:], n2i_in.ap())
            n2i = cst.tile([128, 128], BF16, tag="n2i", name="n2i")
            nc.vector.tensor_copy(n2i[:], n2is[:])

            ones_s = cst.tile([128, 1], BF16, tag="oness", name="ones_s")
            nc.sync.dma_start(ones_s[:], ones_in.ap())
            ones_a = cst.tile([128, 1], BF16, tag="onesa", name="ones_a")
            nc.vector.tensor_copy(ones_a[:], ones_s[:])

            xt = cst.tile([128, EXT_W], BF16, tag="xt", name="xt")
            for jc in range(NCHUNK):
                c0, c1 = jc * 1024, min((jc + 1) * 1024, EXT_W)
                nc.sync.dma_start(xt[:, c0:c1], xt_in.ap()[:, c0:c1])

            NS = cst.tile([128, NSLOT], F32, tag="NS", name="NS")
            css = cst.tile([1, NCHUNK * 1024], F32, tag="css", name="css")

            # hoist the ACT table load: dummy Exp/Square on a const tile at
            # t=0 so the ~1.3us ACT_TABLE_LOAD overlaps the input DMAs
            warm0 = cst.tile([128, 1], F32, tag="warm0", name="warm0")
            nc.gpsimd.memset(warm0[:], 0.0)
            warm1 = cst.tile([128, 1], F32, tag="warm1", name="warm1")
            nc.scalar.activation(warm1[:], warm0[:], AF.Exp, bias=-MHN, scale=64.0)
            nc.scalar.activation(warm1[:], warm0[:], AF.Square, bias=0.75)

            state = {}

            def emit_sims(g):
                lo = g["pieces"][0]["lo"]
                lhsT = xt[:, lo: lo + 128]
                for p in g["pieces"]:
                    L = p["b"] - p["a"]
                    ps_t = psd.tile([128, 1024], F32, tag="ps",
                                    name=f"ps_{p['a']}")
                    for s0 in range(0, L, 512):
                        s1 = min(s0 + 512, L)
                        has_diag = p["first"] and s0 == 0
                        nc.tensor.matmul(
                            ps_t[:, s0:s1], lhsT,
                            xt[:, p["a"] + s0: p["a"] + s1],
                            start=True, stop=not has_diag,
                        )
                        if has_diag:
                            nc.tensor.matmul(
                                ps_t[:, 0:128], n2i[:], i128[:],
                                start=False, stop=True, skip_group_check=True,
                            )
                    state[("ps", p["r"], p["a"])] = ps_t

            def emit_squares(g):
                eng = g["eng"]
                ga = g["segs"][0]["a"] if g["segs"][0]["kind"] == "pref" else None
                main = g["segs"][-1]
                if main["kind"] == "main":
                    u2h = sbh.tile([128, 2048], F16, tag="u2h",
                                   name=f"u2h{main['slot']}")
                    state[("u2h", id(g))] = u2h
                if eng == "act" and main["kind"] == "main":
                    u2f = sbq.tile([128, 2048], F32, tag="u2f",
                                   name=f"u2f{main['slot']}")
                    state[("u2f", id(g))] = u2f
                for p in g["pieces"]:
                    ps_t = state[("ps", p["r"], p["a"])]
                    L = p["b"] - p["a"]
                    s0 = 0
                    if p["first"]:
                        u2a = sbq.tile([128, 256], F32, tag="u2a",
                                       name=f"u2a{p['r']}")
                        nc.scalar.activation(
                            u2a[:, 0:PREF], ps_t[:, 0:PREF], AF.Square,
                            bias=0.75,
                        )
                        state[("u2a", id(g))] = u2a
                        s0 = PREF
                    off = p["a"] + s0 - main["a"]
                    w = L - s0
                    if w <= 0:
                        continue
                    if eng == "act":
                        u2f = state[("u2f", id(g))]
                        nc.scalar.activation(
                            u2f[:, off: off + w], ps_t[:, s0:L], AF.Square,
                            bias=0.75,
                        )
                    else:
                        u2h = state[("u2h", id(g))]
                        uh = sbh.tile([128, 1024], F16, tag="uh",
                                      name=f"uh{p['a']}")
                        nc.vector.tensor_scalar(
                            uh[:, 0:w], ps_t[:, s0:L], 0.75, None, OP.add
                        )
                        mul_eng = nc.vector if eng == "dve" else nc.gpsimd
                        mul_eng.tensor_tensor(
                            u2h[:, off: off + w], uh[:, 0:w], uh[:, 0:w],
                            op=OP.mult,
                        )
                for p in g["pieces"]:
                    state.pop(("ps", p["r"], p["a"]))

            def emit_exp(g):
                ga, gb = g["pieces"][0]["a"], g["pieces"][-1]["b"]
                E = sbe.tile([128, 2048], BF16, tag="E", name=f"E{id(g)}")
                for seg in g["segs"]:
                    if seg["kind"] == "pref":
                        src = state.pop(("u2a", id(g)))[:, 0:PREF]
                    else:
                        w = seg["b"] - seg["a"]
                        if g["eng"] == "act":
                            src = state.pop(("u2f", id(g)))[:, 0:w]
                        else:
                            src = state.pop(("u2h", id(g)))[:, 0:w]
                    nc.scalar.activation(
                        E[:, seg["a"] - ga: seg["b"] - ga], src, AF.Exp,
                        bias=-MHN, scale=64.0,
                        accum_out=NS[:, seg["slot"]: seg["slot"] + 1],
                    )
                state[("E", id(g))] = E


            def emit_colsum(g, gi):
                if not g["cs_subs"]:
                    state.pop(("E", id(g)))
                    return
                ga = g["pieces"][0]["a"]
                E = state.pop(("E", id(g)))
                for (g0, g1) in g["cs_subs"]:
                    jc = g0 // 1024
                    cs_t = state.get(("cs", jc))
                    if cs_t is None:
                        cs_t = psc.tile([1, 1024], F32, tag="cs",
                                        name=f"cs{jc}")
                        state[("cs", jc)] = cs_t
                    bank = g0 // 512
                    nc.tensor.matmul(
                        cs_t[0:1, g0 - jc * 1024: g1 - jc * 1024],
                        ones_a[:],
                        E[:, g0 - ga: g1 - ga],
                        start=BANK_FIRST[bank] == (gi, g0),
                        stop=BANK_LAST[bank] == (gi, g0),
                        skip_group_check=True,
                    )

            def emit_evac(jc):
                cs_t = state.pop(("cs", jc), None)
                if cs_t is None:
                    return
                c0 = jc * 1024
                if EVAC_ENG[jc] == "act":
                    nc.scalar.copy(css[0:1, c0: c0 + 1024], cs_t[0:1, :])
                else:
                    nc.vector.tensor_copy(css[0:1, c0: c0 + 1024], cs_t[0:1, :])

            def emit_pw(r_idx):
                i = LOCAL_ROWS[r_idx]
                lo = EXT_OFF + i * 128
                pw = psd.tile([128, 1024], F32, tag="ps", name=f"pw{r_idx}")
                nc.tensor.matmul(
                    pw[:, 0:256], xt[:, lo: lo + 128],
                    xt[:, lo - 64: lo + 192], start=True, stop=True,
                )
                pws = cst.tile([128, 256], F32, tag=f"pws{r_idx}",
                               name=f"pws{r_idx}")
                nc.vector.tensor_copy(pws[:], pw[:, 0:256])
                nc.sync.dma_start(pw_out.ap()[r_idx, :, :], pws[:])

            # pipelined emission: sims(G), squares(G), exp(G-1), colsum(G-2)
            # window sims spread mid-stream (chunks 0-4 landed by step 12)
            evac_due = {}
            for jc, gi in CHUNK_LAST.items():
                evac_due.setdefault(gi, []).append(jc)
            for step in range(NGRP + 2):
                if step < NGRP:
                    emit_sims(GROUPS[step])
                    emit_squares(GROUPS[step])
                if 12 <= step < 20:
                    emit_pw(step - 12)
                if 1 <= step < NGRP + 1:
                    emit_exp(GROUPS[step - 1])
                if step >= 2:
                    gi = step - 2
                    emit_colsum(GROUPS[gi], gi)
                    for jc in sorted(evac_due.get(gi, [])):
                        emit_evac(jc)

            # ---------------- writeback ----------------
            nc.sync.dma_start(stats_out.ap()[:], NS[:])
            nc.sync.dma_start(cs_out.ap()[:], css[:])

    nc.compile()
    return nc


def _get_prog():
    global _PROG
    if _PROG is None:
        _PROG = _build()
    return _PROG


def _prepare_inputs(embeddings, labels):
    x = np.asarray(embeddings, dtype=np.float32)
    lab = np.asarray(labels)
    assert x.shape == (B, D) and lab.shape == (B,)

    perm = np.argsort(lab, kind="stable")
    xs = x[perm]
    ls = lab[perm]

    _, inv_idx, counts = np.unique(ls, return_inverse=True, return_counts=True)
    cnt_row = counts[inv_idx]
    valid_sorted = (cnt_row >= 2) & (B - cnt_row >= 1)
    assert counts.max() <= 64, "window of 256 requires class size <= 64"

    e = xs / np.linalg.norm(xs.astype(np.float64), axis=1, keepdims=True).astype(
        np.float32
    )
    eT = np.ascontiguousarray(e.T)

    ident = np.eye(128, dtype=bfloat16)
    n2i = (-2.0 * np.eye(128)).astype(bfloat16)
    ones128 = np.ones((128, 1), dtype=bfloat16)

    ext_src = (np.arange(EXT_W) - EXT_OFF) % B
    in_maps = []
    for k in range(NCORES):
        sh = 512 * k
        rot_cols = (ext_src + sh) % B
        xt = np.ascontiguousarray(eT[:, rot_cols]).astype(bfloat16)
        in_maps.append(
            {"xt": xt, "i128": ident, "n2i": n2i, "ones128": ones128}
        )
    return in_maps, valid_sorted, ls


def _epilogue(results, valid_sorted, ls):
    NEG = np.zeros(B)
    CRv = np.zeros(B)
    PSv = np.zeros(B)
    cs_lo, cs_hi = 192, EXT_OFF + (35 + 32) * 128  # [192, 8640)
    ccols = np.arange(cs_lo, cs_hi)
    prow = np.arange(128)
    for k in range(NCORES):
        st = np.asarray(results[k]["stats"], dtype=np.float64)
        cs = np.asarray(results[k]["cs"], dtype=np.float64)[0]
        pw = np.asarray(results[k]["pw"], dtype=np.float64)
        sh = 512 * k
        for g in GROUPS:
            i = g["pieces"][0]["i"]
            rows = (sh + i * 128 + prow) % B
            for seg in g["segs"]:
                NEG[rows] += st[:, seg["slot"]]
        np.add.at(NEG, (ccols - EXT_OFF + sh) % B, cs[ccols])
        for r_idx, i in enumerate(LOCAL_ROWS):
            rows = (sh + i * 128 + prow) % B
            lr_rows = ls[rows]
            wcols = (sh + i * 128 - 64 + np.arange(256)) % B
            eq = lr_rows[:, None] == ls[wcols][None, :]
            eq[prow, 64 + prow] = False
            s = pw[r_idx]
            u2w = (s + 0.75) ** 2
            v2 = (s - 0.75) ** 2
            CRv[rows] += np.where(eq, np.exp(64.0 * u2w - MHN), 0.0).sum(axis=1)
            PSv[rows] += np.where(eq, np.exp(64.0 * v2 - MHP), 0.0).sum(axis=1)

    neg = np.maximum(NEG - CRv, 1e-250)
    with np.errstate(divide="ignore", invalid="ignore"):
        negterm = np.log(neg) + MHN
        posterm = np.log(np.maximum(PSv, 1e-250)) + MHP
    per_row = np.logaddexp(0.0, negterm + posterm)
    per_row = np.where(valid_sorted, per_row, 0.0)
    count = int(valid_sorted.sum())
    return np.float32(per_row.sum() / max(count, 1))


def kernel(embeddings, labels, _trace=False):
    nc = _get_prog()
    in_maps, valid_sorted, ls = _prepare_inputs(embeddings, labels)
    res = run_bass_kernel_spmd(
        nc, in_maps, core_ids=list(range(NCORES)), trace=_trace
    )
    loss = _epilogue(res.results, valid_sorted, ls)
    if _trace:
        return loss, res
    return loss
